# revision 17
# baseline (speedup 1.0000x reference)
"""Hexagonal conv2d (HConv2D) Trainium2 kernel.

Math (verified vs the jax reference to 2.5e-7):
  out[n, 2i,   w, f] = relu(b + a[2i] + bb[2i+1] + c[2i+2])        (w-aligned)
  out[n, 2i+1, w, f] = relu(b + a'[2i+1][w] + c[2i+2][w-1])
with per-input-row 1D convs over Cin=128 -> F=256:
  a[r][w]  = k01.x[r,w]   + k02.x[r,w+1]
  a'[r][w] = k01.x[r,w-1] + k02.x[r,w]
  bb[r][w] = k10.x[r,w-1] + k11.x[r,w] + k12.x[r,w+1]
  c[r][w]  = k21.x[r,w]   + k22.x[r,w+1]
where krc = kernel[r, c] : [Cin, F].  KEY: c[2i+2] is shared between the
even and odd output rows (odd reads it shifted by one column), so it is
computed ONCE on the PE and added into both outputs on the Vector
engine: 9 big tap-matmuls per output-row-pair instead of 11 (18% PE cut).
The odd w=0 seam column (c[-1] = k22.x[0]) is batched: one N=64 matmul
per (image, F-chunk) fed from a tiny dedicated seam-source input tensor
(so it never waits on the big x DMA).

Distribution: data-parallel over batch (16 -> 8 cores x 2 images). Host
transposes x to [n, c, h, w] (c on partitions = contraction dim), pads
h/w with zeros, casts to bf16. PSUM tiles are [Fchunk=128, 4 rows, 128 w]
(one bank, N=512).

Trace-tuned pipeline. HW facts this encodes: PE streams N=512 matmuls
back-to-back at ~220ns with LDWEIGHTS fully hidden (~127us floor);
ScalarE ACTIVATE costs ~712ns@FD512 / ~1.14us@FD1024 regardless of
dtype, so one merged activation per block keeps ScalarE (~1.8us/blk incl
the c-copy) under the PE rate (~2.0us/blk); DVE does the three PSUM adds
(~1.55us/blk); DMAs issued from Scalar/GpSimd queues drain through a
single SDMA engine (~26GB/s) so ALL bulk DMAs go on the Sync queue -
whose ring is FIFO, so output-DMA completions lag ~20us behind the input
burst at the start: deep SBUF rings (fo/ob/c_sb) absorb that lag, else
ScalarE stalls on DMA-completion sems and the whole pipeline convoys
onto a cold-clocked PE; psum rings sized ps_c=3/ps_o=2/ps_e=3 (8 banks)
to decouple the PE from the c-copy latency chain; output is bf16 (halves
write traffic; host casts to fp32); 7 warmup matmuls bridge the PE HAM
clock-gate from ~8us to first-data (~10.5us); the last block runs a
per-parity tail so only a half-size TT_e/act_e/dma_e chain trails the
final matmul.
"""

import numpy as np
import ml_dtypes

import concourse.bacc as bacc
import concourse.bass as bass
import concourse.mybir as mybir
import concourse.tile as tile
from concourse.bass_utils import run_bass_kernel_spmd

N_CORES = 8
NPC = 2            # images per core
H = W = 128
C = 128            # input channels
F = 256            # filters
HP, WP = H + 1, W + 2
HB = 4             # out-row-pairs per psum tile (4 pairs -> N=512)
NHB = (H // 2) // HB

# tap weight order: kernel[r][c] for these (r, c)
TAP_RC = [(0, 1), (0, 2), (1, 0), (1, 1), (1, 2), (2, 1), (2, 2)]

BF16 = mybir.dt.bfloat16
F32 = mybir.dt.float32


def _build():
    nc = bacc.Bacc(
        "TRN2", target_bir_lowering=False, debug=False, num_devices=N_CORES
    )
    xt = nc.dram_tensor("xt", (NPC, C, HP, WP), BF16, kind="ExternalInput").ap()
    wt = nc.dram_tensor("wt", (C, 7, F), BF16, kind="ExternalInput").ap()
    bs = nc.dram_tensor("bs", (F // 2, 2), F32, kind="ExternalInput").ap()
    # seam sources: xm[c, n, i] = x[n, c, 2i+2, 0] (i = 0..63)
    xm = nc.dram_tensor("xm", (C, NPC, 64), BF16, kind="ExternalInput").ap()
    ot = nc.dram_tensor(
        "ot", (NPC, F, 2, H // 2, W), BF16, kind="ExternalOutput"
    ).ap()

    with tile.TileContext(nc) as tc:
        with (
            tc.tile_pool(name="const", bufs=1) as const,
            tc.tile_pool(name="xpool", bufs=1) as xpool,
            tc.tile_pool(name="psum", bufs=2, space="PSUM") as psum,
            tc.tile_pool(name="osb", bufs=6) as osb,
        ):
            xs = [
                xpool.tile([C, HP, WP], BF16, name=f"xs{n}", tag=f"xs{n}")
                for n in range(NPC)
            ]
            # All bulk DMAs on the Sync HWDGE queue (16-way SDMA split).
            # Issue order = landing order: first x rows, first F-chunk
            # weights, seam sources, second F-chunk, bias, rest of x.
            nc.sync.dma_start(out=xs[0][:, 0:9, :], in_=xt[0, :, 0:9, :])
            w_sb = const.tile([C, 7, F], BF16, name="w_sb")
            nc.sync.dma_start(out=w_sb[:, :, 0:128], in_=wt[:, :, 0:128])
            xm_sb = const.tile([C, NPC, 64], BF16, name="xm_sb")
            nc.sync.dma_start(out=xm_sb[:], in_=xm[:])
            nc.sync.dma_start(out=w_sb[:, :, 128:256], in_=wt[:, :, 128:256])
            b_sb = const.tile([F // 2, 2], F32, name="b_sb")
            nc.sync.dma_start(out=b_sb[:], in_=bs[:])
            chunks = {
                0: [(9, 25), (25, 49), (49, 81), (81, 113), (113, HP)],
                1: [(0, 33), (33, 66), (66, 99), (99, HP)],
            }
            for n in range(NPC):
                for h0, h1 in chunks[n]:
                    nc.sync.dma_start(
                        out=xs[n][:, h0:h1, :], in_=xt[n, :, h0:h1, :]
                    )

            # Warm the PE HAM clock-gate during the input-DMA wait: dummy
            # matmuls on a zeroed scratch tile fill the otherwise-idle
            # window between engine preamble and first data, so the real
            # matmuls start at 2.4GHz instead of the cold 1.2GHz.
            warm_sb = const.tile([128, 512], BF16, name="warm_sb")
            nc.vector.memset(warm_sb[:], 0.0)
            ps_w = psum.tile([128, 512], F32, name="ps_w", tag="ps_c", bufs=3)
            for _ in range(7):
                nc.tensor.matmul(
                    ps_w[:], warm_sb[:, 0:128], warm_sb[:],
                    start=True, stop=True)

            add = mybir.AluOpType.add
            relu = mybir.ActivationFunctionType.Relu
            for n in range(NPC):
                # Batched odd-row w=0 seam for the whole image: one N=64
                # matmul per F-chunk (s[i] = k22.x[2i+2, 0]) from the
                # dedicated seam-source tile.
                seams = []
                for fj in range(2):
                    fsl = slice(fj * 128, (fj + 1) * 128)
                    ps_s = psum.tile([128, HB, W], F32, name="ps_s", tag="ps_c", bufs=3)
                    nc.tensor.matmul(
                        ps_s[:, 0, 0:64], w_sb[:, 6, fsl],
                        xm_sb[:, n], start=True, stop=True)
                    s_sb = osb.tile(
                        [128, 64, 1], F32, name=f"s{fj}", tag=f"seam{fj}", bufs=2
                    )
                    nc.vector.tensor_copy(s_sb[:], ps_s[:, 0, 0:64])
                    seams.append(s_sb)
                def emit_block(p0, npr, fj, last):
                    """Pairs p0..p0+npr-1 of image n, F-chunk fj.

                    last=True splits the post-chain per parity so only
                    TT_e -> act_e -> dma_e trails the final matmul.
                    """
                    fsl = slice(fj * 128, (fj + 1) * 128)
                    r0 = 2 * p0
                    rE = slice(r0, r0 + 2 * npr - 1, 2)      # rows 2i
                    rO = slice(r0 + 1, r0 + 2 * npr, 2)      # rows 2i+1
                    rC = slice(r0 + 2, r0 + 2 * npr + 1, 2)  # rows 2i+2
                    q = slice(0, npr)

                    # Three PSUM accumulation groups; ps_e last so the
                    # odd-parity DVE adds overlap the ps_e matmuls.
                    # c[2i+2][w] = k21.x[w] + k22.x[w+1], w = 0..127
                    ps_c = psum.tile(
                        [128, HB, W], F32, name="ps_c", tag="ps_c", bufs=3
                    )
                    # odd: a'[2i+1]; w=0 seam column added on DVE
                    ps_o = psum.tile(
                        [128, HB, W], F32, name="ps_o", tag="ps_o", bufs=2
                    )
                    # even: a[2i] + bb[2i+1]
                    ps_e = psum.tile(
                        [128, HB, W], F32, name="ps_e", tag="ps_e", bufs=3
                    )
                    mm = nc.tensor.matmul
                    mm(ps_c[:, q], w_sb[:, 5, fsl], xs[n][:, rC, 1:129],
                       start=True, stop=False)
                    mm(ps_c[:, q], w_sb[:, 6, fsl], xs[n][:, rC, 2:130],
                       start=False, stop=True)
                    mm(ps_o[:, q], w_sb[:, 0, fsl], xs[n][:, rO, 0:128],
                       start=True, stop=False)
                    mm(ps_o[:, q], w_sb[:, 1, fsl], xs[n][:, rO, 1:129],
                       start=False, stop=True)
                    mm(ps_e[:, q], w_sb[:, 0, fsl], xs[n][:, rE, 1:129],
                       start=True, stop=False)
                    mm(ps_e[:, q], w_sb[:, 1, fsl], xs[n][:, rE, 2:130],
                       start=False, stop=False)
                    mm(ps_e[:, q], w_sb[:, 2, fsl], xs[n][:, rO, 0:128],
                       start=False, stop=False)
                    mm(ps_e[:, q], w_sb[:, 3, fsl], xs[n][:, rO, 1:129],
                       start=False, stop=False)
                    mm(ps_e[:, q], w_sb[:, 4, fsl], xs[n][:, rO, 2:130],
                       start=False, stop=True)

                    # DVE cannot read two PSUM operands in one op: stage c
                    # in SBUF (ScalarE), then DVE adds it into both
                    # parities of one bf16 ob tile; odd first so it
                    # overlaps the ps_e matmuls.
                    c_sb = osb.tile(
                        [128, HB, W], F32, name="c_sb", tag="c_sb", bufs=10
                    )
                    nc.scalar.copy(c_sb[:, q], ps_c[:, q])
                    ob = osb.tile(
                        [128, 2, HB, W], BF16, name="ob", tag="ob", bufs=10
                    )
                    nc.vector.tensor_tensor(
                        ob[:, 1, q, 0:1], ps_o[:, q, 0:1],
                        seams[fj][:, p0 : p0 + npr], op=add)
                    nc.vector.tensor_tensor(
                        ob[:, 1, q, 1:128], ps_o[:, q, 1:128],
                        c_sb[:, q, 0:127], op=add)
                    fo = osb.tile(
                        [128, 2, HB, W], BF16, name="fo", tag="fo", bufs=20
                    )
                    if not last:
                        nc.vector.tensor_tensor(
                            ob[:, 0, q], ps_e[:, q], c_sb[:, q], op=add)
                        nc.scalar.activation(
                            fo[:, :, q], ob[:, :, q], relu,
                            bias=b_sb[:, fj : fj + 1],
                        )
                        nc.sync.dma_start(
                            out=ot[n, fsl, :, p0 : p0 + npr, :],
                            in_=fo[:, :, q],
                        )
                    else:
                        # Final block: per-parity tail, even parity in
                        # half-size pieces so the post-matmul chain is
                        # as short as possible.
                        nc.scalar.activation(
                            fo[:, 1, q], ob[:, 1, q], relu,
                            bias=b_sb[:, fj : fj + 1],
                        )
                        nc.sync.dma_start(
                            out=ot[n, fsl, 1, p0 : p0 + npr, :],
                            in_=fo[:, 1, q],
                        )
                        h2 = npr // 2
                        for piece in range(2):
                            psl = slice(piece * h2, (piece + 1) * h2)
                            nc.vector.tensor_tensor(
                                ob[:, 0, psl], ps_e[:, psl],
                                c_sb[:, psl], op=add)
                            nc.scalar.activation(
                                fo[:, 0, psl], ob[:, 0, psl], relu,
                                bias=b_sb[:, fj : fj + 1],
                            )
                            nc.sync.dma_start(
                                out=ot[n, fsl, 0,
                                       p0 + piece * h2 : p0 + (piece + 1) * h2,
                                       :],
                                in_=fo[:, 0, psl],
                            )

                for hb in range(NHB):
                    final_hb = n == NPC - 1 and hb == NHB - 1
                    # Final hb: emit fj=1 first so its output DMA issues
                    # before the last block's tail chain - Sync DMA issues
                    # serialize (~0.6us each) and would otherwise land in
                    # the middle of the trailing act->dma chain.
                    for fj in ((1, 0) if final_hb else (0, 1)):
                        emit_block(hb * HB, HB, fj,
                                   last=final_hb and fj == 0)
    nc.compile()
    return nc


_NC_CACHE = None


def _get_nc():
    global _NC_CACHE
    if _NC_CACHE is None:
        _NC_CACHE = _build()
    return _NC_CACHE


def _prep_core_inputs(x_shard, wt_host, bs_host):
    xp = np.zeros((NPC, C, HP, WP), dtype=ml_dtypes.bfloat16)
    xp[:, :, :H, 1 : 1 + W] = x_shard.transpose(0, 3, 1, 2)
    # xm[c, n, i] = x[n, c, 2i+2, 0] for i=0..63 (row 128 is the zero pad row)
    xm = xp[:, :, 2:129:2, 1].transpose(1, 0, 2)
    return {"xt": xp, "wt": wt_host, "bs": bs_host, "xm": np.ascontiguousarray(xm)}


def kernel(x, kernel, bias):
    x = np.asarray(x, dtype=np.float32)
    kernel = np.asarray(kernel, dtype=np.float32)
    bias = np.asarray(bias, dtype=np.float32)

    wt_host = np.stack(
        [kernel[r, c] for (r, c) in TAP_RC], axis=1
    ).astype(ml_dtypes.bfloat16)  # (C, 7, F)
    bs_host = np.ascontiguousarray(
        bias.reshape(2, F // 2).T
    ).astype(np.float32)  # (128, 2): bs[f, j] = bias[j*128+f]

    nc = _get_nc()
    in_maps = [
        _prep_core_inputs(x[i * NPC : (i + 1) * NPC], wt_host, bs_host)
        for i in range(N_CORES)
    ]
    res = run_bass_kernel_spmd(nc, in_maps, list(range(N_CORES)))

    outs = [res.results[i]["ot"] for i in range(N_CORES)]  # (NPC,F,2,H/2,W)
    full = np.concatenate(outs, axis=0)  # (16, F, 2, H/2, W) bf16
    # out[n, h, w, f] with h = 2*h2 + parity
    out = full.transpose(0, 3, 2, 4, 1).reshape(16, H, W, F)
    return np.ascontiguousarray(out.astype(np.float32))
